# revision 43
# baseline (speedup 1.0000x reference)
"""Causal multi-head attention (B=4, S=2048, H=2048, NH=16) on 8 TRN2 NeuronCores.

Strategy (tensor-parallel over heads + all-to-all reshard), v3:
  - Each core owns 2 heads. Host slices W_attn/b_attn per core, casts to
    bf16, and relayouts x^T / weights so every SBUF load is one big DMA
    ([128, kc, cols] layouts; fp32 accumulation happens in PSUM).
  - Phase A (per batch, 25 emission units): Q^T, K^T AND V^T are all
    produced feature-major with 512-col weight-stationary matmuls (the
    per-matmul fixed cost dominates LDWEIGHTS, so fewer/longer matmuls
    win); V^T is then PE-transposed into the token-major vst layout.
    Transposes are deferred one unit so they never wait on the vector
    drain that produced their input.
  - Phase B (per batch, per head): scores^T = K^T.T @ Q^T on causal
    blocks only, single-bank PSUM score tiles (exp per 512-col block);
    P^T tiles are PV stationary operands, rhs = [V | ones] whose 129th
    column carries the softmax denominator for free. Normalize with
    per-row reciprocal on VectorE. One AllToAll per (batch, head): the
    h0 exchange overlaps h1 compute.
  - Phase C: output projection for this core's 256-token slice. C(b-1)
    runs right after B(b)'s A stretch; B3 has no fillers so all of C2
    plus C3's even-parity half-units cover the last A2A + transpose
    latency; C3's odd halves borrow the idle scores/PV PSUM banks so
    all 8 accumulators stay live across the even/odd split.
  - DMA queue discipline: x tiles + wproj + out-writes trigger on sync,
    weights/consts on gpsimd (collectives stay unblocked there), a2a_in
    writes and the a2a_out DMA-transposes on the scalar queue, emitted
    mid-A-stretch so they never head-of-line block B-phase exps.
  - _dedup_ldweights removes back-to-back reloads of an identical PE
    stationary after tile legalization.

Self-contained: hardcodes all shapes; no file reads.
"""

import numpy as np
import ml_dtypes

import concourse.bacc as bacc
import concourse.tile as tile
import concourse.mybir as mybir
from concourse import bass_utils

BF16 = mybir.dt.bfloat16
F32 = mybir.dt.float32
AF = mybir.ActivationFunctionType

N_CORES = 8
B = 4
S = 2048
H = 2048
NH = 16
HD = 128
HPC = NH // N_CORES          # heads per core = 2
TOK = B * S                  # 8192
KCH = H // 128               # 16 hidden chunks
SC = 512                     # token chunk for projections / q-chunks
TPB_CH = S // SC             # 4 token chunks per batch
QB = S // 128                # 16 q/kv blocks per batch
SCALE = 1.0 / float(np.sqrt(HD))
VSTRIDE = 2 * (HD + 1)       # V storage: per tokblock [Vh0|1|Vh1|1]
TPB = S // N_CORES           # 256 tokens per core per batch after A2A

_CACHE: dict = {}
LAST_RESULT = None
LDW_REMOVED = 0


def _dedup_ldweights(nc):
    """Remove InstLdweights whose stationary operand is identical to the
    immediately preceding PE weight load (no intervening PE instruction
    that could clobber the array, no semaphore waits/updates on the
    duplicate). The PE array keeps its loaded stationary across matmuls,
    so consecutive matmuls sharing a stationary need only one load.
    Runs after tile legalization (which inserts one InstLdweights per
    matmul) and before nc.compile().
    """
    removed = 0
    for blk in nc.main_func.blocks:
        out = []
        last_key = None
        for inst in blk.instructions:
            if isinstance(inst, mybir.InstLdweights):
                si = inst.sync_info
                clean = si is None or (not si.on_wait and not si.on_update)
                ap = inst.ins[0]
                key = (getattr(ap, 'memref', None),
                       getattr(ap, 'offset', None),
                       str(getattr(ap, 'ap', None)),
                       str(getattr(ap, 'dtype', None)),
                       inst.is_transpose, inst.perf_mode,
                       inst.tile_position, inst.tile_size)
                if clean and key == last_key:
                    removed += 1
                    continue
                last_key = key
                out.append(inst)
            elif isinstance(inst, mybir.InstMatmult):
                if inst.is_transpose:
                    last_key = None
                out.append(inst)
            else:
                if getattr(inst, 'engine', None) == mybir.EngineType.PE:
                    last_key = None
                out.append(inst)
        blk.instructions[:] = out
    return removed


def _build():
    nc = bacc.Bacc("TRN2", target_bir_lowering=False, debug=False,
                   num_devices=N_CORES)
    # Host-relayouted inputs: leading dim 128 = SBUF partition.
    xT = nc.dram_tensor("xT", [128, KCH, TOK], BF16, kind="ExternalInput")
    wqkv = nc.dram_tensor("wqkv", [128, KCH, 6 * HD], BF16,
                          kind="ExternalInput")
    wproj = nc.dram_tensor("wproj", [128, KCH, H], BF16,
                           kind="ExternalInput")
    bqk_t = nc.dram_tensor("bqk_t", [128, 6], F32, kind="ExternalInput")
    bproj = nc.dram_tensor("bproj", [1, H], BF16, kind="ExternalInput")
    mask = nc.dram_tensor("mask", [128, 128], BF16, kind="ExternalInput")
    ident = nc.dram_tensor("ident", [128, 128], BF16, kind="ExternalInput")
    # bf16 output (host upcasts): halves the drain DMA and fits the
    # rel-err budget comfortably.
    out = nc.dram_tensor("out", [B * TPB, H], BF16, kind="ExternalOutput")

    with tile.TileContext(nc) as tc:
        with (
            tc.tile_pool(name="const", bufs=1) as constp,
            tc.tile_pool(name="qkp", bufs=8) as qkp,
            tc.tile_pool(name="dram", bufs=1, space="DRAM") as dram,
            tc.tile_pool(name="xTp", bufs=7) as xTp,
            tc.tile_pool(name="vTp", bufs=2) as vTp,
            tc.tile_pool(name="psAC", bufs=4, space="PSUM") as psAC,
            tc.tile_pool(name="psS", bufs=2, space="PSUM") as psS,
            tc.tile_pool(name="psPV", bufs=2, space="PSUM") as psPV,
            tc.tile_pool(name="ptP", bufs=8) as ptP,
            tc.tile_pool(name="an4P", bufs=1) as an4P,
            tc.tile_pool(name="recP", bufs=4) as recP,
            tc.tile_pool(name="atP", bufs=4) as atP,
            tc.tile_pool(name="outP", bufs=2) as outP,
        ):
            # ---- resident weights / consts -------------------------------
            # Weights stream on the gpsimd queue while x tiles stream on
            # sync: the two queues fan out over the same HW DMA engines, so
            # the first compute unit waits max(wq, x0) instead of the sum.
            wq_t = constp.tile([128, KCH * 6 * HD], BF16, name="wq_t")
            # Geometric piece-loads: the first compute unit's kc=0 matmul
            # only waits for a single-chunk piece (~0.5us of data), and
            # later kc chunks stream in just ahead of their use.
            kc0 = 0
            for nkc in (1, 1, 2, 4, 8):
                nc.gpsimd.dma_start(
                    wq_t[:, kc0 * 6 * HD:(kc0 + nkc) * 6 * HD],
                    wqkv[:, kc0:kc0 + nkc, :])
                kc0 += nkc

            xt_tiles = [None] * (4 * TPB_CH * B)   # (t, quarter) -> tile

            def load_x(t, q):
                xtile = xTp.tile([128, 4 * SC], BF16, name="xt")
                nc.sync.dma_start(
                    xtile[:],
                    xT[:, q * 4:(q + 1) * 4, t * SC:(t + 1) * SC])
                xt_tiles[4 * t + q] = xtile

            load_x(0, 0)
            load_x(0, 1)
            load_x(0, 2)
            load_x(0, 3)

            mask_sb = constp.tile([128, 128], BF16, name="mask_sb")
            nc.gpsimd.dma_start(mask_sb[:], mask[:])
            ones_sb = constp.tile([1, 128], BF16, name="ones_sb")
            nc.vector.memset(ones_sb[:], 1.0)
            bqkt_sb = constp.tile([128, 6], F32, name="bqkt_sb")
            nc.gpsimd.dma_start(bqkt_sb[:], bqk_t[:])
            bproj_sb = constp.tile([1, H], BF16, name="bproj_sb")
            nc.gpsimd.dma_start(bproj_sb[:], bproj[:])
            ident_sb = constp.tile([128, 128], BF16, name="ident_sb")
            nc.gpsimd.dma_start(ident_sb[:], ident[:])

            # V stores: 2 persistent slots; ones columns memset once.
            vst = [constp.tile([128, QB * VSTRIDE], BF16, name=f"vst{i}")
                   for i in range(2)]
            nc.vector.memset(vst[0][:], 1.0)
            nc.vector.memset(vst[1][:], 1.0)

            # W_proj resident, loaded late (first used in B3). Triggers go
            # on the sync queue interleaved between x-tile triggers so the
            # x-slot WAR waits naturally stagger the 2.1MB chunks instead
            # of letting them all race the startup-critical x loads.
            wp_t = constp.tile([128, KCH * H], BF16, name="wp_t")

            def load_wproj(c):
                nc.sync.dma_start(
                    wp_t[:, c * 4 * H:(c + 1) * 4 * H],
                    wproj[:, c * 4:(c + 1) * 4, :])

            qk_store = [None] * B
            # One collective per (batch, head): head h's exchange triggers
            # after h's PV groups finish, so its network time overlaps the
            # other head's compute, and each burst is half the bytes. Flat
            # [shard, payload] layout keeps per-peer chunks contiguous.
            a2a_in = [[dram.tile([N_CORES, TPB * HD], BF16,
                                 name=f"cc_in{b}h{h}") for h in range(HPC)]
                      for b in range(B)]
            a2a_out = [[dram.tile([N_CORES, TPB * HD], BF16,
                                  name=f"cc_out{b}h{h}") for h in range(HPC)]
                       for b in range(B)]

            def a2a_view(t):
                return t.rearrange("s (t c) -> (s t) c", c=HD)

            at_w = [None] * B

            # Deferred V^T->V transposes: each entry transposes one vt tile
            # ([128 d, 512 tok]) into its vst token-blocks. Flushed at the
            # start of a LATER unit so the PE never waits on the vector
            # drain that produced vt.
            vt_pending = []

            def _flush_one():
                vt, vslot, tloc, vh = vt_pending.pop(0)
                for tb in range(4):
                    psw = psAC.tile([128, SC], F32, name="psa")
                    tv = psw[:, 0:64].bitcast(BF16)
                    nc.tensor.transpose(
                        tv, vt[:, tb * 128:(tb + 1) * 128], ident_sb[:])
                    base = (tloc * 4 + tb) * VSTRIDE + vh * (HD + 1)
                    nc.vector.tensor_copy(vslot[:, base:base + HD], tv)

            def flush_vt():
                # only flush entries at least one full unit old, so the PE
                # transpose never waits on the vector drain that made vt
                if len(vt_pending) >= 2:
                    _flush_one()

            def flush_vt_all():
                while vt_pending:
                    _flush_one()

            # ---- phase A as a unit generator -----------------------------
            def phase_a_units(b):
                """Yield 25 emission units for batch b's QKV projection.
                V is produced feature-major (V^T) with the same 512-col
                weight-stationary matmuls as Q/K, then PE-transposed into
                the token-major vst layout (129-col PV moving operand
                with its ones denominator columns kept intact)."""
                qk_store[b] = [qkp.tile([128, S], BF16, name="qkt")
                               for _ in range(4)]
                vslot = vst[b % 2]
                for tloc in range(TPB_CH):
                    t = b * TPB_CH + tloc
                    for vh in range(2):      # v_h0, v_h1 feature-major
                        def u_v(t=t, tloc=tloc, vh=vh, vslot=vslot):
                            flush_vt()
                            # prefetch next tchunk (crossing into the next
                            # batch at tloc==3; slot WAR throttles timing)
                            if t + 1 < B * TPB_CH:
                                load_x(t + 1, 2 * vh)
                                load_x(t + 1, 2 * vh + 1)
                            ps = psAC.tile([128, SC], F32, name="psa")
                            for kc in range(KCH):
                                xth = xt_tiles[4 * t + kc // 4]
                                nc.tensor.matmul(
                                    ps[:],
                                    wq_t[:, kc * 6 * HD + (4 + vh) * HD:
                                         kc * 6 * HD + (5 + vh) * HD],
                                    xth[:, (kc % 4) * SC:(kc % 4 + 1) * SC],
                                    start=(kc == 0), stop=(kc == KCH - 1))
                            vt = vTp.tile([128, SC], BF16, name="vt")
                            nc.vector.tensor_scalar_add(
                                vt[:], ps[:], bqkt_sb[:, 4 + vh:5 + vh])
                            vt_pending.append((vt, vslot, tloc, vh))
                        yield u_v
                    for ob in range(4):      # q_h0, q_h1, k_h0, k_h1
                        def u_qk(t=t, tloc=tloc, ob=ob):
                            flush_vt()
                            ps = psAC.tile([128, SC], F32, name="psa")
                            for kc in range(KCH):
                                xth = xt_tiles[4 * t + kc // 4]
                                nc.tensor.matmul(
                                    ps[:],
                                    wq_t[:, kc * 6 * HD + ob * 128:
                                         kc * 6 * HD + (ob + 1) * 128],
                                    xth[:, (kc % 4) * SC:(kc % 4 + 1) * SC],
                                    start=(kc == 0), stop=(kc == KCH - 1))
                            nc.vector.tensor_scalar_add(
                                qk_store[b][ob][:, tloc * SC:(tloc + 1) * SC],
                                ps[:], bqkt_sb[:, ob:ob + 1])
                        yield u_qk
                yield flush_vt_all      # drain the batch's last vts

            # ---- phase C as a unit generator -----------------------------
            def phase_c_transposes(b):
                """Two wide DMA-transposes bring the received buffer in as
                a^T: partition = hidden-within-head-half, free =
                shard*TPB + token."""
                at_w[b] = []
                for h in range(HPC):
                    atile = atP.tile([128, S], BF16, name="at")
                    # scalar queue (DMA transpose needs a HWDGE engine):
                    # this trigger waits on the A2A-done semaphore; on the
                    # sync queue it would head-of-line block the x-tile
                    # load triggers emitted after it. A2A(b,h) completes
                    # well before the next B phase's exps, so no HOL here.
                    nc.scalar.dma_start(
                        atile[:],
                        a2a_view(a2a_out[b][h]),
                        transpose=True)
                    at_w[b].append(atile)

            def _c_matmuls(b, oc, tb, ps, hcs, start):
                for n, hc in enumerate(hcs):
                    nc.tensor.matmul(
                        ps[:],
                        at_w[b][hc % 2][:, (hc // 2) * TPB + tb * 128:
                                        (hc // 2) * TPB + (tb + 1) * 128],
                        wp_t[:, hc * H + oc * SC:hc * H + (oc + 1) * SC],
                        start=(start and n == 0), stop=False)

            def _c_finish(b, oc, tb, ps):
                nc.tensor.matmul(
                    ps[:], ones_sb[:], bproj_sb[:, oc * SC:(oc + 1) * SC],
                    start=False, stop=True)
                ot = outP.tile([128, SC], BF16, name="ot")
                nc.vector.tensor_copy(ot[:], ps[:])
                nc.sync.dma_start(
                    out[b * TPB + tb * 128:b * TPB + (tb + 1) * 128,
                        oc * SC:(oc + 1) * SC],
                    ot[:])

            def phase_c_units(b):
                """Yield 8 units: output projection for this core's token
                slice of batch b. Out-writes trigger on the sync queue
                (idle by this point) so the gpsimd queue stays clear for
                collectives."""
                for oc in range(4):
                    for tb in range(TPB // 128):
                        def u_c(oc=oc, tb=tb):
                            ps = psAC.tile([128, SC], F32, name="psa")
                            _c_matmuls(b, oc, tb, ps, range(KCH), True)
                            _c_finish(b, oc, tb, ps)
                        yield u_c

            def run_c_split(b):
                """Run batch b's C phase as even/odd-parity half-unit
                waves: all 8 even-hc halves (needing only the h0
                transpose) run while the h1 A2A is still in flight; the
                odd halves follow on the same psum tiles. The B phases
                are over, so the scores/PV PSUM banks are free — borrow
                them to keep all 8 accumulators live at once."""
                evens = list(range(0, KCH, 2))
                odds = list(range(1, KCH, 2))
                pools = [(psAC, "psa")] * 4 + [(psS, "pss")] * 2 + \
                        [(psPV, "ppv")] * 2
                tiles = []
                for u in range(8):
                    oc, tb = u // 2, u % 2
                    pool, nm = pools[u]
                    ps = pool.tile([128, SC], F32, name=nm)
                    tiles.append(ps)
                    _c_matmuls(b, oc, tb, ps, evens, True)
                for u in range(8):
                    oc, tb = u // 2, u % 2
                    _c_matmuls(b, oc, tb, tiles[u], odds, False)
                    _c_finish(b, oc, tb, tiles[u])

            # ---- phase B with interleaved filler units -------------------
            def phase_b(b, fillers):
                """Attention for batch b (both heads) + its AllToAll.
                Runs one filler unit between each score group and its PV
                group so the PE stream stays fed while ScalarE exps."""
                vslot = vst[b % 2]
                for h in range(HPC):
                    qt = qk_store[b][h]
                    kt = qk_store[b][2 + h]
                    for qc in range(4):
                        pts = []
                        for pr in range(2 * (qc + 1)):
                            pt2 = ptP.tile([128, 2 * SC], BF16, name="pt")
                            for j in range(2):
                                kb = 2 * pr + j
                                col0 = max(0, kb * 128 - qc * SC)
                                psb = psS.tile([128, SC], F32, name="pss")
                                nc.tensor.matmul(
                                    psb[:, col0:SC],
                                    kt[:, kb * 128:(kb + 1) * 128],
                                    qt[:, qc * SC + col0:(qc + 1) * SC],
                                    start=True, stop=True)
                                nc.scalar.activation(
                                    pt2[:, j * SC + col0:(j + 1) * SC],
                                    psb[:, col0:SC],
                                    AF.Exp, scale=SCALE)
                                if kb >= 4 * qc:
                                    nc.vector.tensor_mul(
                                        pt2[:, j * SC + col0:
                                            j * SC + col0 + 128],
                                        pt2[:, j * SC + col0:
                                            j * SC + col0 + 128],
                                        mask_sb[:])
                            pts.append(pt2)
                        if fillers:
                            fillers.pop(0)()
                        an4 = an4P.tile([128, 4 * HD], BF16, name="an4")
                        for qb in range(4):
                            qg = qc * 4 + qb
                            po = psPV.tile([128, SC], F32,
                                           name="ppv")[:, 0:HD + 1]
                            for kb in range(qg + 1):
                                pr, j = divmod(kb, 2)
                                vbase = kb * VSTRIDE + h * (HD + 1)
                                nc.tensor.matmul(
                                    po[:],
                                    pts[pr][:, j * SC + qb * 128:
                                            j * SC + (qb + 1) * 128],
                                    vslot[:, vbase:vbase + HD + 1],
                                    start=(kb == 0), stop=(kb == qg))
                            rec = recP.tile([128, 1], F32, name="rec")
                            nc.vector.reciprocal(rec[:], po[:, HD:HD + 1])
                            nc.vector.tensor_scalar_mul(
                                an4[:, qb * 128:(qb + 1) * 128],
                                po[:, 0:HD], rec[:])
                        # scalar queue: its next ACT starts later than this
                        # write's dependency resolves, so no head-of-line
                        # block, and the sync queue stays free for x loads.
                        nc.scalar.dma_start(
                            a2a_view(a2a_in[b][h])[qc * SC:(qc + 1) * SC, :]
                            .rearrange("(qb q) c -> q qb c", qb=4),
                            an4[:])
                    nc.gpsimd.collective_compute(
                        "AllToAll",
                        mybir.AluOpType.bypass,
                        replica_groups=[list(range(N_CORES))],
                        ins=[a2a_in[b][h].opt()],
                        outs=[a2a_out[b][h].opt()],
                    )

            def run_units(units):
                for u in units:
                    u()

            # ---- software-pipelined emission -----------------------------
            # A0(+wp spread) [B0*A1] A1rest [B1*C0] A2 [B2*C1] A3 [B3*C2]
            # C3. Each B(b)'s fillers are the C units of batch b-1 (their
            # A2A + transposes completed during the preceding A stretch),
            # so only C3 remains after B3 — a ~35us tail instead of three
            # C phases. phase_c_transposes(b) is emitted right after
            # phase_b(b): the scalar queue reaches it early and only the
            # A2A-done semaphore gates it.
            a0 = list(phase_a_units(0))
            for i, u in enumerate(a0):
                u()
                if i in (11, 15, 19, 23):
                    load_wproj((i - 11) // 4)
            # T(b) is emitted a few units INTO the following A stretch: the
            # scalar queue is idle there (A units have no scalar work), so
            # the transpose trigger sits waiting on the A2A(b,*)-done
            # semaphores without head-of-line blocking any B-phase exps.
            a1 = list(phase_a_units(1))
            phase_b(0, a1[:8])
            run_units(a1[8:12])
            phase_c_transposes(0)
            run_units(a1[12:])
            a2 = list(phase_a_units(2))
            phase_b(1, a2[:8])
            run_units(a2[8:12])
            phase_c_transposes(1)
            run_units(a2[12:])
            run_units(list(phase_c_units(0)))
            a3 = list(phase_a_units(3))
            phase_b(2, a3[:8])
            run_units(a3[8:12])
            phase_c_transposes(2)
            run_units(a3[12:])
            run_units(list(phase_c_units(1)))
            c2 = list(phase_c_units(2))
            phase_b(3, [])
            phase_c_transposes(3)
            run_units(c2)
            run_c_split(3)

    global LDW_REMOVED
    LDW_REMOVED = _dedup_ldweights(nc)
    nc.compile()
    return nc


def _get_nc():
    if "nc" not in _CACHE:
        _CACHE["nc"] = _build()
    return _CACHE["nc"]


def kernel(hidden_states, W_attn, b_attn, W_proj, b_proj):
    global LAST_RESULT
    bf = ml_dtypes.bfloat16
    x = np.asarray(hidden_states, dtype=np.float32).reshape(TOK, H)
    xb = x.astype(bf)
    # [128, kc, t] layout: xT3[p, kc, t] = x[t, kc*128+p]
    xT3 = np.ascontiguousarray(
        xb.reshape(TOK, KCH, 128).transpose(2, 1, 0))
    Wa = np.asarray(W_attn, dtype=np.float32)
    ba = np.asarray(b_attn, dtype=np.float32)
    Wp = np.asarray(W_proj, dtype=np.float32).astype(bf)
    wp3 = np.ascontiguousarray(Wp.reshape(KCH, 128, H).transpose(1, 0, 2))
    bp = np.asarray(b_proj, dtype=np.float32).reshape(1, H).astype(bf)
    mask = np.triu(np.ones((128, 128), dtype=np.float32)).astype(bf)
    identity = np.eye(128, dtype=np.float32).astype(bf)

    in_maps = []
    for c in range(N_CORES):
        h0 = c * HPC
        cols = []
        for part in range(3):          # q, k, v feature slices
            cols.append(np.arange(part * H + h0 * HD,
                                  part * H + (h0 + HPC) * HD))
        cols = np.concatenate(cols)    # 768 column indices
        wq = Wa[:, cols].astype(bf)
        wq3 = np.ascontiguousarray(
            wq.reshape(KCH, 128, 6 * HD).transpose(1, 0, 2))
        # per-partition bias for the 6 Q^T/K^T/V^T feature blocks
        bqk_t = np.ascontiguousarray(
            ba[cols].reshape(6, 128).T).astype(np.float32)
        in_maps.append({
            "xT": xT3,
            "wqkv": wq3,
            "wproj": wp3,
            "bqk_t": bqk_t,
            "bproj": bp,
            "mask": mask,
            "ident": identity,
        })

    nc = _get_nc()
    res = bass_utils.run_bass_kernel_spmd(
        nc, in_maps, core_ids=list(range(N_CORES)))
    LAST_RESULT = res

    full = np.empty((B, S, H), dtype=np.float32)
    for c in range(N_CORES):
        r = np.asarray(res.results[c]["out"], dtype=np.float32)
        for b in range(B):
            full[b, c * TPB:(c + 1) * TPB, :] = r[b * TPB:(b + 1) * TPB, :]
    return full

